# revision 30
# baseline (speedup 1.0000x reference)
"""Trainium2 Bass kernel for nn_DiagonalRefine (8-core SPMD).

Math: the reference extracts the main diagonal of feat [2,256,512,512],
runs grouped-conv1d(k=3,g=8)+GELU, dense-conv1d(k=3)+GELU on it, embeds
the result back on the diagonal of a zero image, then depthwise 3x3-blurs
it. The blur of a diagonal-only image is zero outside 5 diagonals:
  out[i, i+d] for d in [-2..2], built from 9 per-channel blur weights and
  sig[i-1], sig[i], sig[i+1].

Sharding: rows are split 8 ways (64 rows/core). Each core receives only
its data dependency — the 70-entry diagonal neighborhood (halo 3 each
side) of feat, host-extracted via a numpy diagonal view — and returns
only its algorithmic output: the 5-wide band [B,C,64,5]. The host
unshards by scattering the bands onto the diagonals of a zero canvas
(every other output element is structurally zero for all inputs).

On device (v2): PE path in bf16 (f32 matmul runs as 2 LDW+MM passes;
bf16 is single-pass with fast weight load). Matmuls are batched over the
batch dim (rhs [128, 2, cols]) so each weight slab loads once: 6 conv1 +
12 conv2. w1 stores only the used co-half (block-diag). Constants ride 3
parallel DMA paths (hot w1+masks on ACT ring, w2 on SWDGE, diag+f32
scalars on SP ring). Exact GELU on ScalarE (table preloaded at boot via
a dummy activation), masks+band on VectorE with 3 products offloaded to
ScalarE (Copy with per-partition scale), one merged band-store DMA.

Wait-slot note: every instruction gets at most ONE sync wait; observer
ops (dummy matmuls / copy / memsets) make each engine see a DMA
semaphore before any real consumer.
"""

import sys

for _p in ("/opt/trn_rl_repo",):
    if _p not in sys.path:
        sys.path.append(_p)

import numpy as np

import concourse.bass as bass
import concourse.mybir as mybir
from concourse import tile
from concourse.bass_utils import run_bass_kernel_spmd
from bass_rust import add_dep_helper

# ---- problem geometry (hardcoded; see spec) --------------------------------
B = 2
C = 256
L = 512
NCORES = 8
RB = L // NCORES          # 64 rows per core
T = RB + 6                # 70 diag positions (halo 3 each side)
M = T - 2                 # 68 mid positions
S = M - 2                 # 66 sig positions
BAND = 4 * RB * 5         # per-partition band elems, quarters q=(h,b)
FP32 = mybir.dt.float32
BF16 = mybir.dt.bfloat16

# bf16 weight table per-partition layout (hot1 | hot2 | w2 DMA chunks)
K0_OFF = 0                 # conv1 k=0 slabs: h*128 (co half h only)
MH2_OFF = 256              # h-mask replicated per b: [2*M] = 136
K1_OFF = MH2_OFF + 2 * M   # 392: conv1 k=1 slabs
K2_OFF = K1_OFF + 256      # 648: conv1 k=2 slabs
W2_OFF = K2_OFF + 256      # 904: (k*2+ci_h)*256 + h*128
W16_FREE = W2_OFF + 6 * C  # 2440
W1_OFFS = (K0_OFF, K1_OFF, K2_OFF)

# f32 scalar table per-partition layout
WB_OFF = 0                 # 18: (h)*9 + ki*3 + kj
B1_OFF = 18                # 2
B2_OFF = 20                # 2
MS2_OFF = 22               # s-mask replicated per b: [2*S] = 132
CF32_FREE = MS2_OFF + 2 * S  # 154

_cache = {}


def _build_nc():
    nc = bass.Bass()
    dvecp = nc.declare_dram_parameter("dvec", [128 * 4 * T], BF16, isOutput=False)
    w16p = nc.declare_dram_parameter("w16", [128 * W16_FREE], BF16, isOutput=False)
    cf32p = nc.declare_dram_parameter("cf32", [128 * CF32_FREE], FP32, isOutput=False)
    outp = nc.declare_dram_parameter("out", [128 * BAND], BF16, isOutput=True)

    mul = mybir.AluOpType.mult
    add = mybir.AluOpType.add
    GELU = mybir.ActivationFunctionType.Gelu
    COPY = mybir.ActivationFunctionType.Copy

    with tile.TileContext(nc) as tc:
        with (
            tc.tile_pool(name="const", bufs=1) as cpool,
            tc.tile_pool(name="work", bufs=2) as wpool,
            tc.tile_pool(name="band", bufs=2) as bpool,
            tc.tile_pool(name="mpsum", bufs=2, space=bass.MemorySpace.PSUM) as mpool,
            tc.tile_pool(name="spsum", bufs=2, space=bass.MemorySpace.PSUM) as spool,
            tc.tile_pool(name="dpsum", bufs=1, space=bass.MemorySpace.PSUM) as dpool,
        ):
            # boot: pull the Gelu ACT-table load off the critical path
            zscr = cpool.tile([1, 2], FP32, tag="zscr")
            boot = nc.vector.memset(zscr[:], 0.0)

            w16 = cpool.tile([128, W16_FREE], BF16, tag="w16")
            cf32 = cpool.tile([128, CF32_FREE], FP32, tag="cf32")
            diagall = cpool.tile([128, 4 * T], BF16, tag="diagall")

            # hot consts on the ACT HWDGE ring, chunked so conv1 k=0 can
            # start before the k=1/2 slabs land (same ring pipelines)
            hot1dma = nc.scalar.dma_start(
                w16[:, 0:K1_OFF], bass.AP(w16p, 0, [[W16_FREE, 128], [1, K1_OFF]])
            )
            hot2dma = nc.scalar.dma_start(
                w16[:, K1_OFF:W2_OFF],
                bass.AP(w16p, K1_OFF, [[W16_FREE, 128], [1, W2_OFF - K1_OFF]]),
            )
            # dummy Gelu: forces the ACT table load to run at ~boot time
            nc.scalar.activation(zscr[0:1, 1:2], zscr[0:1, 0:1], GELU)
            # w2 slabs on the SWDGE path (parallel ring)
            w2dma = nc.gpsimd.dma_start(
                w16[:, W2_OFF:W16_FREE],
                bass.AP(w16p, W2_OFF, [[W16_FREE, 128], [1, 6 * C]]),
            )
            # diag + f32 scalars on the SP HWDGE ring
            ddma = nc.sync.dma_start(
                diagall[:], bass.AP(dvecp, 0, [[4 * T, 128], [1, 4 * T]])
            )
            cdma32 = nc.sync.dma_start(
                cf32[:], bass.AP(cf32p, 0, [[CF32_FREE, 128], [1, CF32_FREE]])
            )

            # PSUM tiles (dedicated banks) + dummy-observer bank
            mps = [mpool.tile([128, 2 * M], FP32, tag="mp", name=f"mp{h}") for h in range(2)]
            sps = [spool.tile([128, 2 * S], FP32, tag="sp", name=f"sp{h}") for h in range(2)]
            dps = dpool.tile([128, 2], FP32, tag="dps")

            scratch = cpool.tile([1, 1], FP32, tag="scratch")
            with tc.high_priority():
                # PE observes hot-const sem before first real matmul
                nc.tensor.matmul(dps[0:1, 0:1], w16[:, 0:1], w16[:, 0:1],
                                 start=True, stop=True, skip_group_check=True)
                # ACT observes cf32 sem
                nc.scalar.copy(scratch[:], cf32[0:1, 0:1])
                # DVE observes hot + cf32 sems (dedicated scratch: no WAR)
                vscr = cpool.tile([1, 2], FP32, tag="vscr")
                vobs1 = nc.vector.memset(vscr[0:1, 0:1], 0.0)
                add_dep_helper(vobs1.ins, hot1dma.ins, reason="DVE observes hot sem")
                vobs2 = nc.vector.memset(vscr[0:1, 1:2], 0.0)
                add_dep_helper(vobs2.ins, cdma32.ins, reason="DVE observes cf32 sem")

            def wslab1(k, h):
                s = W1_OFFS[k] + h * 128
                return w16[:, s:s + 128]

            def wslab2(k, ci_h, h):
                s = W2_OFF + (k * 2 + ci_h) * C + h * 128
                return w16[:, s:s + 128]

            mh2_bc = w16[:, MH2_OFF:MH2_OFF + 2 * M]
            ms2_bc = cf32[:, MS2_OFF:MS2_OFF + 2 * S]

            def wb(h, ki, kj):
                s = WB_OFF + h * 9 + ki * 3 + kj
                return cf32[:, s:s + 1]

            # ---- conv1 (grouped, block-diag): batched over b, k-major so
            # the k=0 matmuls run off hot1 while hot2 is still in flight ----
            rhss = [diagall[:, h * 2 * T:(h + 1) * 2 * T].rearrange(
                "p (b t) -> p b t", b=2) for h in range(2)]
            for k in range(3):
                for h in range(2):
                    nc.tensor.matmul(
                        mps[h][:], wslab1(k, h), rhss[h][:, :, k:k + M],
                        start=(k == 0), stop=(k == 2),
                        skip_group_check=True,
                    )
            halls = []
            for h in range(2):
                hall = wpool.tile([128, 2 * M], BF16, tag="hall", name=f"hall{h}")
                nc.scalar.activation(hall[:], mps[h][:], GELU,
                                     bias=cf32[:, B1_OFF + h:B1_OFF + h + 1])
                nc.vector.tensor_mul(hall[:], hall[:], mh2_bc)
                halls.append(hall)

            # PE observes w2 sem before conv2
            nc.tensor.matmul(dps[0:1, 1:2], w16[:, W2_OFF:W2_OFF + 1],
                             w16[:, W2_OFF:W2_OFF + 1],
                             start=True, stop=True, skip_group_check=True)

            # ---- conv2 (dense) + gelu + mask + band, per output half h -----
            # band layout is d-plane major (col = d*256 + h*128 + b*64 + i)
            # so every DVE plane write is a flat contiguous [p, 128] op.
            # VectorE is the sole bandall writer (one-sem store DMA); ScalarE
            # assists with the two cross-products. Each op introduces at most
            # ONE new cross-engine dep (Tile emits explicit same-engine waits
            # and reorders queues, so a second new dep breaks the 1-wait ISA
            # limit).
            bandall = bpool.tile([128, BAND], BF16, tag="bandall")

            def bvd(d, h):
                s = d * 256 + h * 128
                return bandall[:, s:s + 128].rearrange("p (b i) -> p b i", b=2)

            last_acts = []
            last_bands = []
            for h in range(2):
                for k in range(3):
                    for ci_h in range(2):
                        last_mm = nc.tensor.matmul(
                            sps[h][:], wslab2(k, ci_h, h),
                            halls[ci_h][:].rearrange("p (b m) -> p b m", b=2)[:, :, k:k + S],
                            start=(k == 0 and ci_h == 0),
                            stop=(k == 2 and ci_h == 1),
                        )
                sig = wpool.tile([128, 2 * S], FP32, tag="sig", name=f"sig{h}")
                nc.scalar.activation(sig[:], sps[h][:], GELU,
                                     bias=cf32[:, B2_OFF + h:B2_OFF + h + 1])
                nc.vector.tensor_mul(sig[:], sig[:], ms2_bc)

                sigv = sig[:].rearrange("p (b s) -> p b s", b=2)
                s0 = sigv[:, :, 0:RB]      # sig[i-1]
                s1 = sigv[:, :, 1:RB + 1]  # sig[i]
                s2 = sigv[:, :, 2:RB + 2]  # sig[i+1]

                # ScalarE cross-products for d=1 / d=3 second terms
                tmpa = bpool.tile([128, 2 * RB], FP32, tag="tmpa")
                tmpb = bpool.tile([128, 2 * RB], FP32, tag="tmpb")
                tav = tmpa[:].rearrange("p (b i) -> p b i", b=2)
                tbv = tmpb[:].rearrange("p (b i) -> p b i", b=2)
                nc.scalar.activation(tav, s1, COPY, scale=wb(h, 1, 2))
                last_act = nc.scalar.activation(tbv, s2, COPY, scale=wb(h, 2, 1))
                last_acts.append(last_act)

                # VectorE: five planes, v2-proven dep order
                tmp = bpool.tile([128, 2 * RB], FP32, tag="tmp")
                tmp2 = bpool.tile([128, 2 * RB], FP32, tag="tmp2")
                tv = tmp[:].rearrange("p (b i) -> p b i", b=2)
                t2v = tmp2[:].rearrange("p (b i) -> p b i", b=2)
                nc.vector.tensor_scalar_mul(bvd(0, h), s0, wb(h, 0, 2))
                nc.vector.tensor_scalar_mul(bvd(4, h), s2, wb(h, 2, 0))
                nc.vector.tensor_scalar_mul(t2v, s0, wb(h, 0, 0))
                nc.vector.scalar_tensor_tensor(tv, s1, wb(h, 1, 1), t2v, mul, add)
                nc.vector.scalar_tensor_tensor(bvd(2, h), s2, wb(h, 2, 2), tv, mul, add)
                nc.vector.scalar_tensor_tensor(bvd(1, h), s0, wb(h, 0, 1), tav, mul, add)
                last_band = nc.vector.scalar_tensor_tensor(
                    bvd(3, h), s1, wb(h, 1, 0), tbv, mul, add)
                last_bands.append(last_band)

            # merged band store (waits only the DVE band sem)
            outdma = nc.scalar.dma_start(
                bass.AP(outp, 0, [[BAND, 128], [1, BAND]]), bandall[:]
            )

            # ---- tail nop ladders: bring each sequencer's observed clock
            # current one semaphore at a time, so the final drains never
            # carry more than one wait. (The scheduler may hoist these —
            # sem coverage still works from any position.)
            def ladder(eng, deps):
                for dinst in deps:
                    n = eng.nop()
                    add_dep_helper(n.ins, dinst.ins, reason="tail clock catch-up")
            tail = [hot1dma, hot2dma, w2dma, ddma, cdma32, outdma,
                    last_bands[1], last_acts[1], last_mm]
            ladder(nc.sync, tail)
            ladder(nc.scalar, tail)
            ladder(nc.gpsimd, tail)
            ladder(nc.vector, tail)
            ladder(nc.tensor, tail)
    return nc


def _prep_shared(w1, b1, w2, b2, w_blur):
    """Pack weights into the bf16 slab table (w1 used-half slabs + w2 dense
    slabs) and the f32 scalar table (blur taps, biases). Masks are filled
    per-core."""
    bf16 = mybir.dt.np(BF16)
    w16 = np.zeros((128, W16_FREE), np.float32)
    # w1: [co, 32ci, 3k]; block-diag groups of 32; only co-half h kept
    w1s = np.zeros((3, 2, 128, 128), np.float32)  # [k, h, ci_l, co_l]
    for co in range(C):
        g = co // 32
        h = g // 4
        cil0 = (g * 32) % 128
        w1s[:, h, cil0:cil0 + 32, co - h * 128] = w1[co].T
    for k in range(3):
        w16[:, W1_OFFS[k]:W1_OFFS[k] + 256] = \
            w1s[k].transpose(1, 0, 2).reshape(128, 256)
    # w2 dense: [ci_l, (k, ci_h), co(256)] = w2[co, ci_h*128+ci_l, k]
    w2r = w2.transpose(1, 2, 0).reshape(2, 128, 3, C).transpose(1, 2, 0, 3)
    w16[:, W2_OFF:W2_OFF + 6 * C] = w2r.reshape(128, 6 * C)
    cf = np.zeros((128, CF32_FREE), np.float32)
    cf[:, WB_OFF:WB_OFF + 18] = \
        w_blur.reshape(2, 128, 9).transpose(1, 0, 2).reshape(128, 18)
    cf[:, B1_OFF:B1_OFF + 2] = b1.reshape(2, 128).T
    cf[:, B2_OFF:B2_OFF + 2] = b2.reshape(2, 128).T
    return w16.astype(bf16), cf


def _prep_core(dfull, w16, cf, g):
    """dfull: [B, C, L] main diagonal of feat. Build this core's inputs:
    dvec [128, 4T] bf16 (quarters q=(h,b), zero-padded halo), masked
    copies of the weight/scalar tables."""
    bf16 = mybir.dt.np(BF16)
    base = g * RB
    dv = np.zeros((128, 4, T), np.float32)
    lo = max(0, base - 3)
    hi = min(L, base + RB + 3)
    a0 = lo - (base - 3)
    n = hi - lo
    seg = dfull[:, :, lo:hi].reshape(B, 2, 128, n)
    for b in range(B):
        for h in range(2):
            dv[:, 2 * h + b, a0:a0 + n] = seg[b, h]
    mh = np.ones(M, np.float32)
    ms = np.ones(S, np.float32)
    if g == 0:
        mh[0:2] = 0.0
        ms[0] = 0.0
    if g == NCORES - 1:
        mh[M - 2:M] = 0.0
        ms[S - 1] = 0.0
    w16g = w16.copy()
    w16g[:, MH2_OFF:MH2_OFF + 2 * M] = np.tile(mh, 2).astype(bf16)
    cfg = cf.copy()
    cfg[:, MS2_OFF:MS2_OFF + 2 * S] = np.tile(ms, 2)
    return dv.astype(bf16).reshape(128, 4 * T).ravel(), w16g.ravel(), cfg.ravel()


def _run(inputs, trace=False, **kw):
    feat = np.asarray(inputs["feat"], np.float32)
    w16, cf = _prep_shared(
        np.asarray(inputs["w1"], np.float32), np.asarray(inputs["b1"], np.float32),
        np.asarray(inputs["w2"], np.float32), np.asarray(inputs["b2"], np.float32),
        np.asarray(inputs["w_blur"], np.float32),
    )
    dfull = feat.diagonal(0, 2, 3)  # [B, C, L] view, no copy
    in_maps = []
    for g in range(NCORES):
        dv, w16g, cfg = _prep_core(dfull, w16, cf, g)
        in_maps.append({"dvec": dv, "w16": w16g, "cf32": cfg})
    if "nc" not in _cache:
        _cache["nc"] = _build_nc()
    res = run_bass_kernel_spmd(
        _cache["nc"], in_maps, core_ids=list(range(NCORES)), trace=trace, **kw
    )
    _cache["last_result"] = res

    # unshard: scatter each core's 5-diagonal band onto a zero canvas
    full = np.zeros((B, C, L, L), np.float32)
    flat = full.reshape(B, C, L * L)
    dslot = {d: d for d in range(5)}  # plane order is plain d-major
    for g in range(NCORES):
        base = g * RB
        # [p, slot, h, b, i], plane order [0,4,1,2,3]
        band = np.asarray(res.results[g]["out"]).astype(np.float32) \
            .reshape(128, 5, 2, 2, RB)
        for b in range(B):
            for h in range(2):
                for d in range(5):
                    off = d - 2
                    i0 = max(0, -(base + off))
                    i1 = min(RB, L - base - off)
                    if i0 >= i1:
                        continue
                    start = (base + i0) * (L + 1) + off
                    stop = (base + i1 - 1) * (L + 1) + off + 1
                    flat[b, h * 128:(h + 1) * 128, start:stop:L + 1] = \
                        band[:, dslot[d], h, b, i0:i1]
    return full


def kernel(**inputs):
    return _run(inputs, trace=False)


# revision 33
# speedup vs baseline: 1.1174x; 1.1174x over previous
"""Trainium2 Bass kernel for nn_DiagonalRefine (8-core SPMD).

Math: the reference extracts the main diagonal of feat [2,256,512,512],
runs grouped-conv1d(k=3,g=8)+GELU, dense-conv1d(k=3)+GELU on it, embeds
the result back on the diagonal of a zero image, then depthwise 3x3-blurs
it. The blur of a diagonal-only image is zero outside 5 diagonals:
  out[i, i+d] for d in [-2..2], built from 9 per-channel blur weights and
  sig[i-1], sig[i], sig[i+1].

Sharding: rows are split 8 ways (64 rows/core). Each core receives only
its data dependency — the 70-entry diagonal neighborhood (halo 3 each
side) of feat, host-extracted via a numpy diagonal view — and returns
only its algorithmic output: the 5-wide band [B,C,64,5]. The host
unshards by scattering the bands onto the diagonals of a zero canvas
(every other output element is structurally zero for all inputs).

On device (v4): PE path and post-PSUM path in bf16 (PSUM accumulates
f32; exact GELU reads PSUM in f32). Matmuls batch over the batch dim
(rhs [128, 2, cols]): 6 conv1 + 12 conv2, each weight slab loaded once.
Each DMA ring serializes at ~2us/DMA, so inputs ride exactly one DMA
per path: hot slab table (ACT ring), diag+scalars (SP ring), w2 slabs
(SWDGE). Band planes are built by VectorE from three flat full-width
ScalarE pre-scalings of sig (w11/w12/w21·sig), stored h-major via two
per-h DMAs on separate rings so h0's store overlaps h1's compute.

Sync rules this Tile version imposes: ONE sync wait per instruction;
same-engine deps also cost a wait unless covered by an earlier wait in
that queue; the scheduler reorders queues. So every op introduces at
most one NEW dependency, observers (dummy matmul / copy / memset)
absorb DMA sems first, and tail nop ladders keep the final drains to
single waits.
"""

import sys

for _p in ("/opt/trn_rl_repo",):
    if _p not in sys.path:
        sys.path.append(_p)

import numpy as np

import concourse.bass as bass
import concourse.mybir as mybir
from concourse import tile
from concourse.bass_utils import run_bass_kernel_spmd
from bass_rust import add_dep_helper

# ---- problem geometry (hardcoded; see spec) --------------------------------
B = 2
C = 256
L = 512
NCORES = 8
RB = L // NCORES          # 64 rows per core
T = RB + 6                # 70 diag positions (halo 3 each side)
M = T - 2                 # 68 mid positions
S = M - 2                 # 66 sig positions
BAND = 4 * RB * 5         # per-partition band elems
FP32 = mybir.dt.float32
BF16 = mybir.dt.bfloat16

# bf16 weight table per-partition layout (hot | w2 DMA chunks)
K0_OFF = 0                 # conv1 k=0 slabs: h*128 (co half h only)
MH2_OFF = 256              # h-mask replicated per b: [2*M] = 136
MS2_OFF = MH2_OFF + 2 * M  # 392: s-mask replicated per b: [2*S] = 132
K1_OFF = MS2_OFF + 2 * S   # 524
K2_OFF = K1_OFF + 256      # 780
W2_OFF = K2_OFF + 256      # 1036: (k*2+ci_h)*256 + h*128
W16_FREE = W2_OFF + 6 * C  # 2572
W1_OFFS = (K0_OFF, K1_OFF, K2_OFF)

# dvec (bf16): diag quarters q=(h,b) only
DV_FREE = 4 * T            # 280

# f32 scalar table (tensor_scalar needs f32 scalars; activation biases)
WB_OFF = 0                 # 18: (h)*9 + ki*3 + kj
B1_OFF = 18                # 2
B2_OFF = 20                # 2
CF_FREE = 22

_cache = {}


def _build_nc():
    nc = bass.Bass()
    dvecp = nc.declare_dram_parameter("dvec", [128 * DV_FREE], BF16, isOutput=False)
    w16p = nc.declare_dram_parameter("w16", [128 * W16_FREE], BF16, isOutput=False)
    cf32p = nc.declare_dram_parameter("cf32", [128 * CF_FREE], FP32, isOutput=False)
    outp = nc.declare_dram_parameter("out", [128 * BAND], BF16, isOutput=True)

    mul = mybir.AluOpType.mult
    add = mybir.AluOpType.add
    GELU = mybir.ActivationFunctionType.Gelu
    COPY = mybir.ActivationFunctionType.Copy

    with tile.TileContext(nc) as tc:
        with (
            tc.tile_pool(name="const", bufs=1) as cpool,
            tc.tile_pool(name="work", bufs=2) as wpool,
            tc.tile_pool(name="band", bufs=2) as bpool,
            tc.tile_pool(name="mpsum", bufs=2, space=bass.MemorySpace.PSUM) as mpool,
            tc.tile_pool(name="spsum", bufs=2, space=bass.MemorySpace.PSUM) as spool,
            tc.tile_pool(name="dpsum", bufs=1, space=bass.MemorySpace.PSUM) as dpool,
        ):
            # boot: pull the Gelu ACT-table load off the critical path
            zscr = cpool.tile([1, 2], FP32, tag="zscr")
            nc.vector.memset(zscr[:], 0.0)

            w16 = cpool.tile([128, W16_FREE], BF16, tag="w16")
            dtab = cpool.tile([128, DV_FREE], BF16, tag="dtab")
            cf32 = cpool.tile([128, CF_FREE], FP32, tag="cf32")

            # one DMA per path (rings serialize at ~2us/DMA): hot slabs on
            # the ACT ring, diag on the SP ring, f32 scalars then w2 slabs
            # on the SWDGE ring
            hotdma = nc.scalar.dma_start(
                w16[:, 0:W2_OFF], bass.AP(w16p, 0, [[W16_FREE, 128], [1, W2_OFF]])
            )
            # dummy Gelu: forces the ACT table load to run at ~boot time
            nc.scalar.activation(zscr[0:1, 1:2], zscr[0:1, 0:1], GELU)
            ddma = nc.sync.dma_start(
                dtab[:], bass.AP(dvecp, 0, [[DV_FREE, 128], [1, DV_FREE]])
            )
            cdma32 = nc.gpsimd.dma_start(
                cf32[:], bass.AP(cf32p, 0, [[CF_FREE, 128], [1, CF_FREE]])
            )
            w2dma = nc.gpsimd.dma_start(
                w16[:, W2_OFF:W16_FREE],
                bass.AP(w16p, W2_OFF, [[W16_FREE, 128], [1, 6 * C]]),
            )

            # PSUM tiles (dedicated banks) + dummy-observer bank
            mps = [mpool.tile([128, 2 * M], FP32, tag="mp", name=f"mp{h}") for h in range(2)]
            sps = [spool.tile([128, 2 * S], FP32, tag="sp", name=f"sp{h}") for h in range(2)]
            dps = dpool.tile([128, 2], FP32, tag="dps")

            scratch = cpool.tile([1, 1], FP32, tag="scratch")
            with tc.high_priority():
                # PE observes hot sem before first real matmul
                nc.tensor.matmul(dps[0:1, 0:1], w16[:, 0:1], w16[:, 0:1],
                                 start=True, stop=True, skip_group_check=True)
                # ACT observes the f32-scalar sem (bias source)
                nc.scalar.copy(scratch[:], cf32[0:1, 0:1])
                # DVE observes hot + scalar sems (dedicated scratch: no WAR)
                vscr = cpool.tile([1, 2], FP32, tag="vscr")
                vobs1 = nc.vector.memset(vscr[0:1, 0:1], 0.0)
                add_dep_helper(vobs1.ins, hotdma.ins, reason="DVE observes hot sem")
                vobs2 = nc.vector.memset(vscr[0:1, 1:2], 0.0)
                add_dep_helper(vobs2.ins, cdma32.ins, reason="DVE observes cf32 sem")

            def wslab1(k, h):
                s = W1_OFFS[k] + h * 128
                return w16[:, s:s + 128]

            def wslab2(k, ci_h, h):
                s = W2_OFF + (k * 2 + ci_h) * C + h * 128
                return w16[:, s:s + 128]

            mh2_bc = w16[:, MH2_OFF:MH2_OFF + 2 * M]
            ms2_bc = w16[:, MS2_OFF:MS2_OFF + 2 * S]

            def wb(h, ki, kj):
                s = WB_OFF + h * 9 + ki * 3 + kj
                return cf32[:, s:s + 1]

            # ---- conv1 (grouped, block-diag): batched over b ---------------
            halls = []
            for h in range(2):
                rhs = dtab[:, h * 2 * T:(h + 1) * 2 * T].rearrange(
                    "p (b t) -> p b t", b=2)
                for k in range(3):
                    nc.tensor.matmul(
                        mps[h][:], wslab1(k, h), rhs[:, :, k:k + M],
                        start=(k == 0), stop=(k == 2),
                    )
                hall = wpool.tile([128, 2 * M], BF16, tag="hall", name=f"hall{h}")
                nc.scalar.activation(hall[:], mps[h][:], GELU,
                                     bias=cf32[:, B1_OFF + h:B1_OFF + h + 1])
                nc.vector.tensor_mul(hall[:], hall[:], mh2_bc)
                halls.append(hall)

            # PE observes w2 sem before conv2
            nc.tensor.matmul(dps[0:1, 1:2], w16[:, W2_OFF:W2_OFF + 1],
                             w16[:, W2_OFF:W2_OFF + 1],
                             start=True, stop=True, skip_group_check=True)

            # ---- conv2 (dense) + gelu + mask + band, per output half h -----
            # band is h-major: col = h*640 + d*128 + b*64 + i, so DVE plane
            # writes are flat [p,128] and each h's store is one contiguous
            # chunk. VectorE is the sole bandall writer; ScalarE feeds it
            # three flat full-width scalings of sig (w11/w12/w21-scaled).
            bandall = bpool.tile([128, BAND], BF16, tag="bandall")

            def bvd(d, h):
                s = h * 640 + d * 128
                return bandall[:, s:s + 128].rearrange("p (b i) -> p b i", b=2)

            outdmas = []
            for h in range(2):
                for k in range(3):
                    for ci_h in range(2):
                        last_mm = nc.tensor.matmul(
                            sps[h][:], wslab2(k, ci_h, h),
                            halls[ci_h][:].rearrange("p (b m) -> p b m", b=2)[:, :, k:k + S],
                            start=(k == 0 and ci_h == 0),
                            stop=(k == 2 and ci_h == 1),
                        )
                sig = wpool.tile([128, 2 * S], BF16, tag="sig", name=f"sig{h}")
                nc.scalar.activation(sig[:], sps[h][:], GELU,
                                     bias=cf32[:, B2_OFF + h:B2_OFF + h + 1])
                nc.vector.tensor_mul(sig[:], sig[:], ms2_bc)

                sigv = sig[:].rearrange("p (b s) -> p b s", b=2)
                s0 = sigv[:, :, 0:RB]      # sig[i-1]
                s1 = sigv[:, :, 1:RB + 1]  # sig[i]
                s2 = sigv[:, :, 2:RB + 2]  # sig[i+1]

                # ScalarE: three flat scaled copies of (masked) sig; their
                # shifted slices supply every second term below. Order a,b,c
                # so DVE's first stt (on c) covers all three ACT counts.
                sca = wpool.tile([128, 2 * S], BF16, tag="sca", name=f"sca{h}")
                scb = wpool.tile([128, 2 * S], BF16, tag="scb", name=f"scb{h}")
                scc = wpool.tile([128, 2 * S], BF16, tag="scc", name=f"scc{h}")
                nc.scalar.activation(sca[:], sig[:], COPY, scale=wb(h, 1, 2))
                nc.scalar.activation(scb[:], sig[:], COPY, scale=wb(h, 2, 1))
                last_act = nc.scalar.activation(scc[:], sig[:], COPY, scale=wb(h, 1, 1))
                scav = sca[:].rearrange("p (b s) -> p b s", b=2)
                scbv = scb[:].rearrange("p (b s) -> p b s", b=2)
                sccv = scc[:].rearrange("p (b s) -> p b s", b=2)

                # VectorE: five planes; first stt introduces the ACT dep
                tmp = bpool.tile([128, 2 * RB], BF16, tag="tmp")
                tv = tmp[:].rearrange("p (b i) -> p b i", b=2)
                # d=2 chain: w00*s0 + w11*s1, then + w22*s2
                nc.vector.scalar_tensor_tensor(
                    tv, s0, wb(h, 0, 0), sccv[:, :, 1:RB + 1], mul, add)
                nc.vector.scalar_tensor_tensor(bvd(2, h), s2, wb(h, 2, 2), tv, mul, add)
                nc.vector.tensor_scalar_mul(bvd(0, h), s0, wb(h, 0, 2))
                nc.vector.tensor_scalar_mul(bvd(4, h), s2, wb(h, 2, 0))
                nc.vector.scalar_tensor_tensor(
                    bvd(1, h), s0, wb(h, 0, 1), scav[:, :, 1:RB + 1], mul, add)
                last_band = nc.vector.scalar_tensor_tensor(
                    bvd(3, h), s1, wb(h, 1, 0), scbv[:, :, 2:RB + 2], mul, add)

                # per-h band store: h0 on the ACT ring (overlaps h1 compute),
                # h1 on the SP ring; each waits only the DVE band sem
                eng = nc.scalar if h == 0 else nc.sync
                outdmas.append(eng.dma_start(
                    bass.AP(outp, h * 640, [[BAND, 128], [1, 640]]),
                    bandall[:, h * 640:(h + 1) * 640],
                ))

            # ---- tail nop ladders: bring each sequencer's observed clock
            # current one semaphore at a time, so the final drains never
            # carry more than one wait.
            def ladder(eng, deps):
                for dinst in deps:
                    n = eng.nop()
                    add_dep_helper(n.ins, dinst.ins, reason="tail clock catch-up")
            tail = [hotdma, ddma, cdma32, w2dma, outdmas[0], outdmas[1],
                    last_band, last_act, last_mm]
            ladder(nc.sync, tail)
            ladder(nc.scalar, tail)
            ladder(nc.gpsimd, tail)
            ladder(nc.vector, tail)
            ladder(nc.tensor, tail)
    return nc


def _prep_shared(w1, b1, w2, b2, w_blur):
    """Pack weights into the bf16 slab table (w1 used-half slabs + w2 dense
    slabs) and the bf16 scalar block appended to dvec. Masks are filled
    per-core."""
    w16 = np.zeros((128, W16_FREE), np.float32)
    # w1: [co, 32ci, 3k]; block-diag groups of 32; only co-half h kept
    w1s = np.zeros((3, 2, 128, 128), np.float32)  # [k, h, ci_l, co_l]
    for co in range(C):
        g = co // 32
        h = g // 4
        cil0 = (g * 32) % 128
        w1s[:, h, cil0:cil0 + 32, co - h * 128] = w1[co].T
    for k in range(3):
        w16[:, W1_OFFS[k]:W1_OFFS[k] + 256] = \
            w1s[k].transpose(1, 0, 2).reshape(128, 256)
    # w2 dense: [ci_l, (k, ci_h), co(256)] = w2[co, ci_h*128+ci_l, k]
    w2r = w2.transpose(1, 2, 0).reshape(2, 128, 3, C).transpose(1, 2, 0, 3)
    w16[:, W2_OFF:W2_OFF + 6 * C] = w2r.reshape(128, 6 * C)
    # f32 scalar table
    cf = np.zeros((128, CF_FREE), np.float32)
    cf[:, 0:18] = w_blur.reshape(2, 128, 9).transpose(1, 0, 2).reshape(128, 18)
    cf[:, 18:20] = b1.reshape(2, 128).T
    cf[:, 20:22] = b2.reshape(2, 128).T
    return w16, cf


def _prep_core(dfull, w16, g):
    """dfull: [B, C, L] main diagonal of feat. Build this core's inputs:
    dvec [128, 4T] bf16 (diag quarters q=(h,b)) and the masked table."""
    bf16 = mybir.dt.np(BF16)
    base = g * RB
    dv = np.zeros((128, 4, T), np.float32)
    lo = max(0, base - 3)
    hi = min(L, base + RB + 3)
    a0 = lo - (base - 3)
    n = hi - lo
    seg = dfull[:, :, lo:hi].reshape(B, 2, 128, n)
    for b in range(B):
        for h in range(2):
            dv[:, 2 * h + b, a0:a0 + n] = seg[b, h]
    mh = np.ones(M, np.float32)
    ms = np.ones(S, np.float32)
    if g == 0:
        mh[0:2] = 0.0
        ms[0] = 0.0
    if g == NCORES - 1:
        mh[M - 2:M] = 0.0
        ms[S - 1] = 0.0
    w16g = w16.copy()
    w16g[:, MH2_OFF:MH2_OFF + 2 * M] = np.tile(mh, 2)
    w16g[:, MS2_OFF:MS2_OFF + 2 * S] = np.tile(ms, 2)
    return dv.astype(bf16).reshape(128, DV_FREE).ravel(), w16g.astype(bf16).ravel()


def _run(inputs, trace=False, **kw):
    feat = np.asarray(inputs["feat"], np.float32)
    w16, cf = _prep_shared(
        np.asarray(inputs["w1"], np.float32), np.asarray(inputs["b1"], np.float32),
        np.asarray(inputs["w2"], np.float32), np.asarray(inputs["b2"], np.float32),
        np.asarray(inputs["w_blur"], np.float32),
    )
    dfull = feat.diagonal(0, 2, 3)  # [B, C, L] view, no copy
    cfr = cf.ravel()
    in_maps = []
    for g in range(NCORES):
        dvg, w16g = _prep_core(dfull, w16, g)
        in_maps.append({"dvec": dvg, "w16": w16g, "cf32": cfr})
    if "nc" not in _cache:
        _cache["nc"] = _build_nc()
    res = run_bass_kernel_spmd(
        _cache["nc"], in_maps, core_ids=list(range(NCORES)), trace=trace, **kw
    )
    _cache["last_result"] = res

    # unshard: scatter each core's 5-diagonal band onto a zero canvas
    full = np.zeros((B, C, L, L), np.float32)
    flat = full.reshape(B, C, L * L)
    for g in range(NCORES):
        base = g * RB
        # [p, h, d, b, i]
        band = np.asarray(res.results[g]["out"]).astype(np.float32) \
            .reshape(128, 2, 5, 2, RB)
        for b in range(B):
            for h in range(2):
                for d in range(5):
                    off = d - 2
                    i0 = max(0, -(base + off))
                    i1 = min(RB, L - base - off)
                    if i0 >= i1:
                        continue
                    start = (base + i0) * (L + 1) + off
                    stop = (base + i1 - 1) * (L + 1) + off + 1
                    flat[b, h * 128:(h + 1) * 128, start:stop:L + 1] = \
                        band[:, h, d, b, i0:i1]
    return full


def kernel(**inputs):
    return _run(inputs, trace=False)
